# revision 28
# baseline (speedup 1.0000x reference)
"""Edge-parallel COO SpMM (segment_sum of vals * seq[cols] by sorted rows) on 8 trn2 cores.

out[r] = sum_{e: rows[e]==r} vals[e] * seq[0, cols[e], :]

rows are sorted; core k owns row-windows [98k, 98k+98) (64 rows each).
Per core the edges are split into lo/hi col streams (int16 index limit) and
packed densely into 128-edge tiles per chunk of 4 windows -- no per-window
padding.  Each chunk's tiles are gathered by 4 dma_gather instructions
rotating over 4 SWDGE queues (descriptor generation pipelines across queues,
~2.9ns/desc vs ~8ns on one queue).  A tile may straddle windows; for every
(tile, window) pair we build a selection matrix S[e, r] = vals * (radj == r)
on the vector engine and accumulate PE matmuls S^T @ G into the window's
PSUM slot.  PSUM is flushed to an SBUF stage and streamed to HBM per chunk.
No cross-core communication; the host splits edges and concatenates outputs.
"""

import sys

if "/opt/trn_rl_repo" not in sys.path:
    sys.path.insert(0, "/opt/trn_rl_repo")

import numpy as np

N_NODES = 50000
N_EDGES = 1_250_000
D_FEAT = 64
W = 64              # rows per window
NW = 98             # windows per core
NCORE = 8
CHUNK_W = 4         # windows per chunk
HALF = 32768        # int16 index limit
NQ = 4              # SWDGE queues

_compiled = {}


def _build_kernel(plan):
    from concourse import bass, bacc, mybir
    import concourse.tile as tile

    f32 = mybir.dt.float32
    i16 = mybir.dt.int16

    CLO, CHI, PTOT = plan["CLO"], plan["CHI"], plan["PTOT"]

    nc = bacc.Bacc("TRN2", target_bir_lowering=False, debug=False,
                   num_swdge_queues=NQ)
    seq_t = nc.dram_tensor("seq", [N_NODES, D_FEAT], f32, kind="ExternalInput")
    idxlo_t = nc.dram_tensor("idxlo", [128, CLO * 8], i16, kind="ExternalInput")
    idxhi_t = nc.dram_tensor("idxhi", [128, CHI * 8], i16, kind="ExternalInput")
    radj_t = nc.dram_tensor("radj", [128, PTOT], f32, kind="ExternalInput")
    vals_t = nc.dram_tensor("vals", [128, PTOT], f32, kind="ExternalInput")
    out_t = nc.dram_tensor("out", [64, NW * W], f32, kind="ExternalOutput")

    qcounter = 0

    with tile.TileContext(nc) as tc:
        with (
            tc.tile_pool(name="const", bufs=1) as constp,
            tc.tile_pool(name="meta", bufs=4) as metap,
            tc.tile_pool(name="g", bufs=4) as gp,
            tc.tile_pool(name="s", bufs=3) as sp,
            tc.tile_pool(name="ps", bufs=4, space="PSUM") as psp,
            tc.tile_pool(name="st", bufs=1) as stp,
        ):
            iota_t = constp.tile([128, 8, 64], f32, name="iota")
            iota_emitted = False
            stage = stp.tile([64, NW * W], f32, name="stage")

            # zero-fill every G buffer once: tail gather splits scan fewer
            # than t_cnt*128 slots (num_idxs rounded to 16), so the slots
            # beyond num_idxs of a tile's first use would otherwise hold
            # uninitialized SBUF bits (NaN risk in the S=0 matmul lanes)
            for (half, si), sz in plan["gmax"].items():
                for _ in range(4):
                    gw = gp.tile([128, sz, 64], f32, tag=f"g{half}{si}")
                    nc.vector.memset(gw[:], 0.0)

            for ci, ch in enumerate(plan["chunks"]):
                Pc = ch["Pc"]
                p0 = ch["p0"]
                w0, nw = ch["w0"], ch["nw"]

                # idx loads first so gathers are never queued behind radj/vals
                idxc_h = {}
                for half, idx_t in ((0, idxlo_t), (1, idxhi_t)):
                    hc0 = ch["lo_c0"] if half == 0 else ch["hi_c0"]
                    Th = ch["Tlo"] if half == 0 else ch["Thi"]
                    if Th == 0:
                        continue
                    idxc = metap.tile([128, Th * 8], i16, tag=f"idx{half}")
                    nc.sync.dma_start(
                        out=idxc[:], in_=idx_t[:, hc0 * 8 : (hc0 + Th) * 8]
                    )
                    idxc_h[half] = idxc

                radc = metap.tile([128, Pc], f32, tag="radj")
                valc = metap.tile([128, Pc], f32, tag="vals")
                nc.sync.dma_start(out=radc[:], in_=radj_t[:, p0 : p0 + Pc])
                nc.sync.dma_start(out=valc[:], in_=vals_t[:, p0 : p0 + Pc])

                # gather: up to 4 instructions (2 per stream), rotating queues
                G = {}
                for half in (0, 1):
                    if half not in idxc_h:
                        continue
                    base_lo = 0 if half == 0 else HALF
                    base_hi = HALF if half == 0 else N_NODES
                    idxc = idxc_h[half]
                    splits = ch["splits"][half]  # list of (t_off, t_cnt, ni)
                    for si, (t_off, t_cnt, ni) in enumerate(splits):
                        Gt = gp.tile([128, t_cnt, 64], f32, tag=f"g{half}{si}")
                        nc.gpsimd.dma_gather(
                            Gt[:],
                            seq_t[base_lo:base_hi, :],
                            idxc[:, t_off * 8 : t_off * 8 + ni // 16],
                            ni,
                            ni,
                            D_FEAT,
                            single_packet=False,
                            queue_num=qcounter % NQ,
                        )
                        qcounter += 1
                        G[(half, si)] = Gt

                # iota constant emitted after chunk-0 gathers are dispatched
                # (it shares the gpsimd queue; first S-build is its only user)
                if not iota_emitted:
                    nc.gpsimd.iota(
                        iota_t[:],
                        pattern=[[0, 8], [1, 64]],
                        base=0,
                        channel_multiplier=0,
                        allow_small_or_imprecise_dtypes=True,
                    )
                    iota_emitted = True

                # selection matrices: one [128, 64] column per (tile, window) pair
                S = sp.tile([128, Pc, 64], f32, tag="s")
                for g0 in range(0, Pc, 8):
                    gg = min(8, Pc - g0)
                    nc.vector.tensor_tensor(
                        out=S[:, g0 : g0 + gg, :],
                        in0=iota_t[:, :gg, :],
                        in1=radc[:, g0 : g0 + gg].to_broadcast([128, gg, 64]),
                        op=mybir.AluOpType.is_equal,
                    )
                    nc.vector.tensor_tensor(
                        out=S[:, g0 : g0 + gg, :],
                        in0=S[:, g0 : g0 + gg, :],
                        in1=valc[:, g0 : g0 + gg].to_broadcast([128, gg, 64]),
                        op=mybir.AluOpType.mult,
                    )

                ps = psp.tile([64, CHUNK_W * 64], f32, tag="ps")
                for p, (half, si, pos, j, first, last) in enumerate(ch["pairs"]):
                    nc.tensor.matmul(
                        out=ps[:, j * 64 : (j + 1) * 64],
                        lhsT=S[:, p, :],
                        rhs=G[(half, si)][:, pos, :],
                        start=first,
                        stop=last,
                    )

                nc.vector.tensor_copy(
                    out=stage[:, w0 * 64 : (w0 + nw) * 64], in_=ps[:, : nw * 64]
                )
                nc.sync.dma_start(
                    out=out_t[:, w0 * 64 : (w0 + nw) * 64],
                    in_=stage[:, w0 * 64 : (w0 + nw) * 64],
                )

    nc.compile()
    return nc


def _preprocess(seq, vals, rows, cols):
    rows = np.asarray(rows)
    cols = np.asarray(cols)
    vals = np.asarray(vals)

    # per-core contiguous edge ranges (rows sorted)
    core_starts = np.searchsorted(rows, np.arange(NCORE + 1) * NW * W)

    # small warm-up chunks so the 4 gather queues fill early, and tapered
    # final chunks so the post-gather PE/flush tail is short
    chunk_w0 = (
        [0, 1, 2]
        + list(range(4, NW - 8, CHUNK_W))
        + [NW - 8, NW - 6, NW - 4, NW - 2, NW - 1]
    )
    nchunk = len(chunk_w0)

    # Per core / chunk / stream: edge arrays in row order.
    # ed[k][c][h] = (cols_h, radj_h, vals_h) with radj relative to chunk base.
    ed = [[[None, None] for _ in range(nchunk)] for _ in range(NCORE)]
    for k in range(NCORE):
        s, e = int(core_starts[k]), int(core_starts[k + 1])
        r = rows[s:e] - k * NW * W          # 0 .. NW*W-1
        c = cols[s:e]
        v = vals[s:e]
        # chunk of each edge
        chunk_bounds = np.searchsorted(r, np.array(chunk_w0 + [NW]) * W)
        for ci in range(nchunk):
            cs, ce = int(chunk_bounds[ci]), int(chunk_bounds[ci + 1])
            rr = r[cs:ce] - chunk_w0[ci] * W   # 0 .. CHUNK_W*64-1
            cc = c[cs:ce]
            vv = v[cs:ce]
            m = cc < HALF
            ed[k][ci][0] = (cc[m], rr[m], vv[m])
            ed[k][ci][1] = (cc[~m] - HALF, rr[~m], vv[~m])

    # per (chunk, stream): shared tile count = max over cores (>=1 for lo)
    Tlo = np.zeros(nchunk, np.int64)
    Thi = np.zeros(nchunk, np.int64)
    for ci in range(nchunk):
        nlo = max(len(ed[k][ci][0][0]) for k in range(NCORE))
        nhi = max(len(ed[k][ci][1][0]) for k in range(NCORE))
        Tlo[ci] = max(1, -(-nlo // 128))
        Thi[ci] = -(-nhi // 128)

    lo_c0 = np.concatenate([[0], np.cumsum(Tlo)])
    hi_c0 = np.concatenate([[0], np.cumsum(Thi)])
    CLO, CHI = int(Tlo.sum()), int(Thi.sum())

    # windows spanned per (chunk, stream, tile): union over cores
    # (tile t holds stream edges [128t, 128t+128) of that core's chunk)
    chunks = []
    p0 = 0
    for ci, w0 in enumerate(chunk_w0):
        nw = (chunk_w0[ci + 1] if ci + 1 < nchunk else NW) - w0
        spans = {0: [set() for _ in range(int(Tlo[ci]))],
                 1: [set() for _ in range(int(Thi[ci]))]}
        for k in range(NCORE):
            for h in (0, 1):
                rr = ed[k][ci][h][1]
                for t in range(len(spans[h])):
                    seg = rr[128 * t : 128 * (t + 1)]
                    if len(seg):
                        for j in range(int(seg[0]) // W, int(seg[-1]) // W + 1):
                            spans[h][t].add(j)
        # ensure every window of the chunk gets >=1 matmul
        covered = set()
        for h in (0, 1):
            for t in range(len(spans[h])):
                covered |= spans[h][t]
        for j in range(nw):
            if j not in covered:
                spans[0][0].add(j)

        # instruction splits per stream (2 if big enough, else 1); each split
        # scans num_idxs=ni slots -- tail splits round the max-over-core edge
        # count up to 16 instead of a full 128-tile (descgen time ~ slots)
        splits = {}
        for h, T in ((0, int(Tlo[ci])), (1, int(Thi[ci]))):
            nmax = max(len(ed[k][ci][h][0]) for k in range(NCORE))
            if T == 0:
                splits[h] = []
                continue
            if T >= 8:
                t1 = (T + 1) // 2
                parts = [(0, t1), (t1, T - t1)]
            else:
                parts = [(0, T)]
            out = []
            for t_off, t_cnt in parts:
                if t_off + t_cnt == T:  # tail split
                    ni = max(16, -(-(nmax - 128 * t_off) // 16) * 16)
                else:
                    ni = t_cnt * 128
                out.append((t_off, t_cnt, ni))
            splits[h] = out

        # pair list ordered by window (for PSUM start/stop flags)
        raw = []  # (j, half, tile)
        for h in (0, 1):
            for t, sp_ in enumerate(spans[h]):
                for j in sorted(sp_):
                    raw.append((j, h, t))
        raw.sort()
        pairs = []
        for idx, (j, h, t) in enumerate(raw):
            first = idx == 0 or raw[idx - 1][0] != j
            last = idx == len(raw) - 1 or raw[idx + 1][0] != j
            # locate split buffer and position
            for si, (t_off, t_cnt, _ni) in enumerate(splits[h]):
                if t_off <= t < t_off + t_cnt:
                    pairs.append((h, si, t - t_off, j, first, last))
                    break
        Pc = len(pairs)
        chunks.append(dict(
            w0=w0, nw=nw, p0=p0, Pc=Pc, pairs=pairs, raw=raw,
            lo_c0=int(lo_c0[ci]), hi_c0=int(hi_c0[ci]),
            Tlo=int(Tlo[ci]), Thi=int(Thi[ci]), splits=splits,
        ))
        p0 += Pc
    PTOT = p0

    # pack per-core data
    idx_lo = np.zeros((NCORE, CLO * 128), np.int16)
    idx_hi = np.zeros((NCORE, CHI * 128), np.int16)
    radj_pad = np.full((NCORE, PTOT * 128), -1.0, np.float32)
    vals_pad = np.zeros((NCORE, PTOT * 128), np.float32)
    for k in range(NCORE):
        for ci, ch in enumerate(chunks):
            for h, (c0_, T) in ((0, (ch["lo_c0"], ch["Tlo"])),
                                (1, (ch["hi_c0"], ch["Thi"]))):
                cc = ed[k][ci][h][0]
                n = len(cc)
                if T:
                    idx_arr = idx_lo if h == 0 else idx_hi
                    idx_arr[k, c0_ * 128 : c0_ * 128 + n] = cc.astype(np.int16)
            # per-pair radj/vals columns
            for p, (j, h, t) in enumerate(ch["raw"]):
                _, rr, vv = ed[k][ci][h]
                seg_r = rr[128 * t : 128 * (t + 1)]
                seg_v = vv[128 * t : 128 * (t + 1)]
                n = len(seg_r)
                base = (ch["p0"] + p) * 128
                if n:
                    radj_pad[k, base : base + n] = seg_r - j * W
                    vals_pad[k, base : base + n] = seg_v

    def wrap16(a):
        t = a.reshape(-1, 16).T
        return np.ascontiguousarray(np.tile(t, (8, 1)))

    seq2d = np.ascontiguousarray(np.asarray(seq).reshape(N_NODES, D_FEAT))
    in_maps = []
    for k in range(NCORE):
        in_maps.append({
            "seq": seq2d,
            "idxlo": wrap16(idx_lo[k]),
            "idxhi": wrap16(idx_hi[k]),
            "radj": np.ascontiguousarray(radj_pad[k].reshape(PTOT, 128).T),
            "vals": np.ascontiguousarray(vals_pad[k].reshape(PTOT, 128).T),
        })

    gmax = {}
    for ch in chunks:
        for h in (0, 1):
            for si, (t_off, t_cnt, ni) in enumerate(ch["splits"][h]):
                gmax[(h, si)] = max(gmax.get((h, si), 1), t_cnt)
    plan = dict(CLO=CLO, CHI=CHI, PTOT=PTOT, chunks=chunks, gmax=gmax)
    return plan, in_maps


def kernel(seq, vals, rows, cols, _trace=False):
    from concourse.bass_utils import run_bass_kernel_spmd

    plan, in_maps = _preprocess(seq, vals, rows, cols)

    key = (
        plan["CLO"], plan["CHI"], plan["PTOT"],
        tuple(tuple(ch["pairs"]) for ch in plan["chunks"]),
        tuple(tuple(ch["splits"][h]) for ch in plan["chunks"] for h in (0, 1)),
    )
    if key not in _compiled:
        _compiled[key] = _build_kernel(plan)
    nc = _compiled[key]

    res = run_bass_kernel_spmd(nc, in_maps, core_ids=list(range(NCORE)), trace=_trace)

    outs = []
    for k in range(NCORE):
        o = res.results[k]["out"]                        # [64, 6272]
        outs.append(o.reshape(64, NW, 64).transpose(1, 0, 2).reshape(NW * W, 64))
    full = np.concatenate(outs, axis=0)[:N_NODES]
    out = full[None].astype(np.float32)
    if _trace:
        return out, res
    return out


# revision 29
# speedup vs baseline: 1.1354x; 1.1354x over previous
"""Edge-parallel COO SpMM (segment_sum of vals * seq[cols] by sorted rows) on 8 trn2 cores.

out[r] = sum_{e: rows[e]==r} vals[e] * seq[0, cols[e], :]

rows are sorted; core k owns row-windows [98k, 98k+98) (64 rows each).
Per core the edges are split into lo/hi col streams (int16 index limit) and
packed densely into 128-edge tiles per chunk of 4 windows -- no per-window
padding.  Each chunk's tiles are gathered by 4 dma_gather instructions
rotating over 4 SWDGE queues (descriptor generation pipelines across queues,
~2.9ns/desc vs ~8ns on one queue).  A tile may straddle windows; for every
(tile, window) pair we build a selection matrix S[e, r] = vals * (radj == r)
on the vector engine and accumulate PE matmuls S^T @ G into the window's
PSUM slot.  PSUM is flushed to an SBUF stage and streamed to HBM per chunk.
No cross-core communication; the host splits edges and concatenates outputs.
"""

import sys

if "/opt/trn_rl_repo" not in sys.path:
    sys.path.insert(0, "/opt/trn_rl_repo")

import numpy as np

N_NODES = 50000
N_EDGES = 1_250_000
D_FEAT = 64
W = 64              # rows per window
NW = 98             # windows per core
NCORE = 8
CHUNK_W = 4         # windows per chunk
HALF = 32768        # int16 index limit
NQ = 4              # SWDGE queues

_compiled = {}


def _build_kernel(plan):
    from concourse import bass, bacc, mybir
    import concourse.tile as tile

    f32 = mybir.dt.float32
    i16 = mybir.dt.int16

    CLO, CHI, PTOT = plan["CLO"], plan["CHI"], plan["PTOT"]

    nc = bacc.Bacc("TRN2", target_bir_lowering=False, debug=False,
                   num_swdge_queues=NQ)
    seq_t = nc.dram_tensor("seq", [N_NODES, D_FEAT], f32, kind="ExternalInput")
    idxlo_t = nc.dram_tensor("idxlo", [128, CLO * 8], i16, kind="ExternalInput")
    idxhi_t = nc.dram_tensor("idxhi", [128, CHI * 8], i16, kind="ExternalInput")
    radj_t = nc.dram_tensor("radj", [128, PTOT], f32, kind="ExternalInput")
    vals_t = nc.dram_tensor("vals", [128, PTOT], f32, kind="ExternalInput")
    out_t = nc.dram_tensor("out", [64, NW * W], f32, kind="ExternalOutput")

    qcounter = 0

    with tile.TileContext(nc) as tc:
        with (
            tc.tile_pool(name="const", bufs=1) as constp,
            tc.tile_pool(name="meta", bufs=4) as metap,
            tc.tile_pool(name="g", bufs=4) as gp,
            tc.tile_pool(name="s", bufs=3) as sp,
            tc.tile_pool(name="ps", bufs=4, space="PSUM") as psp,
            tc.tile_pool(name="st", bufs=1) as stp,
        ):
            iota_t = constp.tile([128, 8, 64], f32, name="iota")
            iota_emitted = False
            stage = stp.tile([64, NW * W], f32, name="stage")

            # zero-fill every G buffer once: tail gather splits scan fewer
            # than t_cnt*128 slots (num_idxs rounded to 16), so the slots
            # beyond num_idxs of a tile's first use would otherwise hold
            # uninitialized SBUF bits (NaN risk in the S=0 matmul lanes)
            for (half, si), sz in plan["gmax"].items():
                for _ in range(4):
                    gw = gp.tile([128, sz, 64], f32, tag=f"g{half}{si}")
                    nc.vector.memset(gw[:], 0.0)

            for ci, ch in enumerate(plan["chunks"]):
                Pc = ch["Pc"]
                p0 = ch["p0"]
                w0, nw = ch["w0"], ch["nw"]

                # idx loads first so gathers are never queued behind radj/vals
                idxc_h = {}
                for half, idx_t in ((0, idxlo_t), (1, idxhi_t)):
                    hc0 = ch["lo_c0"] if half == 0 else ch["hi_c0"]
                    Th = ch["Tlo"] if half == 0 else ch["Thi"]
                    if Th == 0:
                        continue
                    idxc = metap.tile([128, Th * 8], i16, tag=f"idx{half}")
                    nc.sync.dma_start(
                        out=idxc[:], in_=idx_t[:, hc0 * 8 : (hc0 + Th) * 8]
                    )
                    idxc_h[half] = idxc

                radc = metap.tile([128, Pc], f32, tag="radj")
                valc = metap.tile([128, Pc], f32, tag="vals")
                nc.sync.dma_start(out=radc[:], in_=radj_t[:, p0 : p0 + Pc])
                nc.sync.dma_start(out=valc[:], in_=vals_t[:, p0 : p0 + Pc])

                # gather: up to 4 instructions (2 per stream), rotating queues
                G = {}
                for half in (0, 1):
                    if half not in idxc_h:
                        continue
                    base_lo = 0 if half == 0 else HALF
                    base_hi = HALF if half == 0 else N_NODES
                    idxc = idxc_h[half]
                    splits = ch["splits"][half]  # list of (t_off, t_cnt, ni)
                    for si, (t_off, t_cnt, ni) in enumerate(splits):
                        Gt = gp.tile([128, t_cnt, 64], f32, tag=f"g{half}{si}")
                        nc.gpsimd.dma_gather(
                            Gt[:],
                            seq_t[base_lo:base_hi, :],
                            idxc[:, t_off * 8 : t_off * 8 + ni // 16],
                            ni,
                            ni,
                            D_FEAT,
                            single_packet=False,
                            queue_num=(qcounter + ci) % NQ,
                        )
                        qcounter += 1
                        G[(half, si)] = Gt

                # iota constant emitted after chunk-0 gathers are dispatched
                # (it shares the gpsimd queue; first S-build is its only user)
                if not iota_emitted:
                    nc.gpsimd.iota(
                        iota_t[:],
                        pattern=[[0, 8], [1, 64]],
                        base=0,
                        channel_multiplier=0,
                        allow_small_or_imprecise_dtypes=True,
                    )
                    iota_emitted = True

                # selection matrices: one [128, 64] column per (tile, window) pair
                S = sp.tile([128, Pc, 64], f32, tag="s")
                for g0 in range(0, Pc, 8):
                    gg = min(8, Pc - g0)
                    nc.vector.tensor_tensor(
                        out=S[:, g0 : g0 + gg, :],
                        in0=iota_t[:, :gg, :],
                        in1=radc[:, g0 : g0 + gg].to_broadcast([128, gg, 64]),
                        op=mybir.AluOpType.is_equal,
                    )
                    nc.vector.tensor_tensor(
                        out=S[:, g0 : g0 + gg, :],
                        in0=S[:, g0 : g0 + gg, :],
                        in1=valc[:, g0 : g0 + gg].to_broadcast([128, gg, 64]),
                        op=mybir.AluOpType.mult,
                    )

                ps = psp.tile([64, CHUNK_W * 64], f32, tag="ps")
                for p, (half, si, pos, j, first, last) in enumerate(ch["pairs"]):
                    nc.tensor.matmul(
                        out=ps[:, j * 64 : (j + 1) * 64],
                        lhsT=S[:, p, :],
                        rhs=G[(half, si)][:, pos, :],
                        start=first,
                        stop=last,
                    )

                nc.vector.tensor_copy(
                    out=stage[:, w0 * 64 : (w0 + nw) * 64], in_=ps[:, : nw * 64]
                )
                nc.sync.dma_start(
                    out=out_t[:, w0 * 64 : (w0 + nw) * 64],
                    in_=stage[:, w0 * 64 : (w0 + nw) * 64],
                )

    nc.compile()
    return nc


def _preprocess(seq, vals, rows, cols):
    rows = np.asarray(rows)
    cols = np.asarray(cols)
    vals = np.asarray(vals)

    # per-core contiguous edge ranges (rows sorted)
    core_starts = np.searchsorted(rows, np.arange(NCORE + 1) * NW * W)

    # small warm-up chunks so the 4 gather queues fill early, and tapered
    # final chunks so the post-gather PE/flush tail is short
    chunk_w0 = (
        [0, 1, 2]
        + list(range(4, NW - 8, CHUNK_W))
        + [NW - 8, NW - 6, NW - 4, NW - 2, NW - 1]
    )
    nchunk = len(chunk_w0)

    # Per core / chunk / stream: edge arrays in row order.
    # ed[k][c][h] = (cols_h, radj_h, vals_h) with radj relative to chunk base.
    ed = [[[None, None] for _ in range(nchunk)] for _ in range(NCORE)]
    for k in range(NCORE):
        s, e = int(core_starts[k]), int(core_starts[k + 1])
        r = rows[s:e] - k * NW * W          # 0 .. NW*W-1
        c = cols[s:e]
        v = vals[s:e]
        # chunk of each edge
        chunk_bounds = np.searchsorted(r, np.array(chunk_w0 + [NW]) * W)
        for ci in range(nchunk):
            cs, ce = int(chunk_bounds[ci]), int(chunk_bounds[ci + 1])
            rr = r[cs:ce] - chunk_w0[ci] * W   # 0 .. CHUNK_W*64-1
            cc = c[cs:ce]
            vv = v[cs:ce]
            m = cc < HALF
            ed[k][ci][0] = (cc[m], rr[m], vv[m])
            ed[k][ci][1] = (cc[~m] - HALF, rr[~m], vv[~m])

    # per (chunk, stream): shared tile count = max over cores (>=1 for lo)
    Tlo = np.zeros(nchunk, np.int64)
    Thi = np.zeros(nchunk, np.int64)
    for ci in range(nchunk):
        nlo = max(len(ed[k][ci][0][0]) for k in range(NCORE))
        nhi = max(len(ed[k][ci][1][0]) for k in range(NCORE))
        Tlo[ci] = max(1, -(-nlo // 128))
        Thi[ci] = -(-nhi // 128)

    lo_c0 = np.concatenate([[0], np.cumsum(Tlo)])
    hi_c0 = np.concatenate([[0], np.cumsum(Thi)])
    CLO, CHI = int(Tlo.sum()), int(Thi.sum())

    # windows spanned per (chunk, stream, tile): union over cores
    # (tile t holds stream edges [128t, 128t+128) of that core's chunk)
    chunks = []
    p0 = 0
    for ci, w0 in enumerate(chunk_w0):
        nw = (chunk_w0[ci + 1] if ci + 1 < nchunk else NW) - w0
        spans = {0: [set() for _ in range(int(Tlo[ci]))],
                 1: [set() for _ in range(int(Thi[ci]))]}
        for k in range(NCORE):
            for h in (0, 1):
                rr = ed[k][ci][h][1]
                for t in range(len(spans[h])):
                    seg = rr[128 * t : 128 * (t + 1)]
                    if len(seg):
                        for j in range(int(seg[0]) // W, int(seg[-1]) // W + 1):
                            spans[h][t].add(j)
        # ensure every window of the chunk gets >=1 matmul
        covered = set()
        for h in (0, 1):
            for t in range(len(spans[h])):
                covered |= spans[h][t]
        for j in range(nw):
            if j not in covered:
                spans[0][0].add(j)

        # instruction splits per stream (2 if big enough, else 1); each split
        # scans num_idxs=ni slots -- tail splits round the max-over-core edge
        # count up to 16 instead of a full 128-tile (descgen time ~ slots)
        splits = {}
        for h, T in ((0, int(Tlo[ci])), (1, int(Thi[ci]))):
            nmax = max(len(ed[k][ci][h][0]) for k in range(NCORE))
            if T == 0:
                splits[h] = []
                continue
            if T >= 8:
                t1 = (T + 1) // 2
                parts = [(0, t1), (t1, T - t1)]
            else:
                parts = [(0, T)]
            out = []
            for t_off, t_cnt in parts:
                if t_off + t_cnt == T:  # tail split
                    ni = max(16, -(-(nmax - 128 * t_off) // 16) * 16)
                else:
                    ni = t_cnt * 128
                out.append((t_off, t_cnt, ni))
            splits[h] = out

        # pair list ordered by window (for PSUM start/stop flags)
        raw = []  # (j, half, tile)
        for h in (0, 1):
            for t, sp_ in enumerate(spans[h]):
                for j in sorted(sp_):
                    raw.append((j, h, t))
        raw.sort()
        pairs = []
        for idx, (j, h, t) in enumerate(raw):
            first = idx == 0 or raw[idx - 1][0] != j
            last = idx == len(raw) - 1 or raw[idx + 1][0] != j
            # locate split buffer and position
            for si, (t_off, t_cnt, _ni) in enumerate(splits[h]):
                if t_off <= t < t_off + t_cnt:
                    pairs.append((h, si, t - t_off, j, first, last))
                    break
        Pc = len(pairs)
        chunks.append(dict(
            w0=w0, nw=nw, p0=p0, Pc=Pc, pairs=pairs, raw=raw,
            lo_c0=int(lo_c0[ci]), hi_c0=int(hi_c0[ci]),
            Tlo=int(Tlo[ci]), Thi=int(Thi[ci]), splits=splits,
        ))
        p0 += Pc
    PTOT = p0

    # pack per-core data
    idx_lo = np.zeros((NCORE, CLO * 128), np.int16)
    idx_hi = np.zeros((NCORE, CHI * 128), np.int16)
    radj_pad = np.full((NCORE, PTOT * 128), -1.0, np.float32)
    vals_pad = np.zeros((NCORE, PTOT * 128), np.float32)
    for k in range(NCORE):
        for ci, ch in enumerate(chunks):
            for h, (c0_, T) in ((0, (ch["lo_c0"], ch["Tlo"])),
                                (1, (ch["hi_c0"], ch["Thi"]))):
                cc = ed[k][ci][h][0]
                n = len(cc)
                if T:
                    idx_arr = idx_lo if h == 0 else idx_hi
                    idx_arr[k, c0_ * 128 : c0_ * 128 + n] = cc.astype(np.int16)
            # per-pair radj/vals columns
            for p, (j, h, t) in enumerate(ch["raw"]):
                _, rr, vv = ed[k][ci][h]
                seg_r = rr[128 * t : 128 * (t + 1)]
                seg_v = vv[128 * t : 128 * (t + 1)]
                n = len(seg_r)
                base = (ch["p0"] + p) * 128
                if n:
                    radj_pad[k, base : base + n] = seg_r - j * W
                    vals_pad[k, base : base + n] = seg_v

    def wrap16(a):
        t = a.reshape(-1, 16).T
        return np.ascontiguousarray(np.tile(t, (8, 1)))

    seq2d = np.ascontiguousarray(np.asarray(seq).reshape(N_NODES, D_FEAT))
    in_maps = []
    for k in range(NCORE):
        in_maps.append({
            "seq": seq2d,
            "idxlo": wrap16(idx_lo[k]),
            "idxhi": wrap16(idx_hi[k]),
            "radj": np.ascontiguousarray(radj_pad[k].reshape(PTOT, 128).T),
            "vals": np.ascontiguousarray(vals_pad[k].reshape(PTOT, 128).T),
        })

    gmax = {}
    for ch in chunks:
        for h in (0, 1):
            for si, (t_off, t_cnt, ni) in enumerate(ch["splits"][h]):
                gmax[(h, si)] = max(gmax.get((h, si), 1), t_cnt)
    plan = dict(CLO=CLO, CHI=CHI, PTOT=PTOT, chunks=chunks, gmax=gmax)
    return plan, in_maps


def kernel(seq, vals, rows, cols, _trace=False):
    from concourse.bass_utils import run_bass_kernel_spmd

    plan, in_maps = _preprocess(seq, vals, rows, cols)

    key = (
        plan["CLO"], plan["CHI"], plan["PTOT"],
        tuple(tuple(ch["pairs"]) for ch in plan["chunks"]),
        tuple(tuple(ch["splits"][h]) for ch in plan["chunks"] for h in (0, 1)),
    )
    if key not in _compiled:
        _compiled[key] = _build_kernel(plan)
    nc = _compiled[key]

    res = run_bass_kernel_spmd(nc, in_maps, core_ids=list(range(NCORE)), trace=_trace)

    outs = []
    for k in range(NCORE):
        o = res.results[k]["out"]                        # [64, 6272]
        outs.append(o.reshape(64, NW, 64).transpose(1, 0, 2).reshape(NW * W, 64))
    full = np.concatenate(outs, axis=0)[:N_NODES]
    out = full[None].astype(np.float32)
    if _trace:
        return out, res
    return out
